# revision 23
# baseline (speedup 1.0000x reference)
"""Multi-head attention on 8 Trainium2 NeuronCores.

Problem: x[4, 2048, 1024], 16 heads x 64 dim.
  qkv = x @ w_qkv; attn = softmax(q k^T / 8); out = (attn v) @ w_out + b_out

Sharding: 8 cores = 4 batches x 2 head-groups (8 heads each).
Each core computes a partial out-projection over its 8 heads' dims;
host sums the two partials per batch and adds the bias.

All SBUF operands are bf16 (halves DMA + SBUF vs the f32r version at the
same PE rate; scale-relative error ~2.6e-3 vs the fp32 reference). PSUM
accumulation stays fp32.

Per-core schedule (one merged pipeline, ScalarE exp is the scarce engine):
  chunks 0-7: qT,kT = (w_qk^T x^T) with d on partitions; v natural [n, 512]
    stored per head pair as [v_even | ones | v_odd] so the fused av matmul
    emits unnormalized out^T rows plus replicated softmax row-sums. After
    each chunk: (a) one full early attention stream (pair 0, ic 0)
    staircases through available j-tiles, (b) up to DEFER more cells run
    scores+exp only, parking exp tiles in SBUF so ScalarE stays busy while
    their av matmuls wait for PSUM accumulator slots.
  main loop, i-chunk-major: for ic, for pair: 16 (scores->exp->av) cells
    (deferred cells replay their parked exp tile, av-only), then the
    DVE reciprocal+mul normalize. Once all 4 pairs finish an ic, that
    i-chunk's out-projection groups are queued and interleaved 1-per-2-cells
    into later streams (only streams whose accumulators sit in psB, so the
    out-proj PSUM tag is free). Output is stored as bf16; the host sums the
    two partials per batch in fp32 and adds the bias.
"""

import numpy as np
import ml_dtypes

import concourse.bacc as bacc
import concourse.mybir as mybir
import concourse.tile as tile
from concourse.bass_utils import run_bass_kernel_spmd

F32 = mybir.dt.float32
BF16 = mybir.dt.bfloat16
FP8 = mybir.dt.float8e4
AF = mybir.ActivationFunctionType
DR = mybir.MatmulPerfMode.DoubleRow

B = 4          # batch
N = 2048       # sequence
DM = 1024      # model dim
NH = 16        # heads
DH = 64        # head dim
G = 2          # head groups (cores per batch)
HPC = NH // G  # heads per core = 8
CW = DH * HPC  # per-core qkv column width = 512

NCH = 512      # phase-1 x^T column chunk (double-chunk: q-granularity = ICH)
ICH = 512      # i (query) chunk (per head; a pair shares [128, 2*ICH])

KT = DM // 128      # 8 contraction tiles over d
MT = 2 * CW // 128  # 8 c-tiles for q|k
NJT = N // 128      # 16 j tiles
NIC = N // ICH      # 4 i chunks
NPAIR = HPC // 2    # 4 head pairs

DEFER = 28     # deferred score+exp cells parked in SBUF during chunks
USE_FP8_QK = True  # q/k projection via fp8e4 DoubleRow matmuls


def build_nc(reps=1, fp8_qk=None):
    if fp8_qk is None:
        fp8_qk = USE_FP8_QK
    nc = bacc.Bacc(None, target_bir_lowering=False, debug=False)

    # xT is host-packed to [chunk, partition, k*NCH] so every chunk load is
    # one fully linear 1 MB DMA
    xT = nc.declare_dram_parameter("xT", [N // NCH, 128, KT * NCH], BF16,
                                   isOutput=False)
    if fp8_qk:
        # fp8 copy of x for the DoubleRow q/k projection: per chunk
        # [128, t(4), pair(2), NCH] with d-slice pairing d = t*256+l*128+p.
        # N=512 of moving data per stationary keeps the (FWL-less) DoubleRow
        # LDWEIGHTS stream just under the matmul stream.
        x8 = nc.declare_dram_parameter(
            "x8", [N // NCH, 128, (KT // 2) * 2 * NCH], FP8, isOutput=False)
        wqk8 = nc.declare_dram_parameter(
            "wqk8", [KT // 2, 128, 2 * 2 * CW], FP8, isOutput=False)
    else:
        wqk = nc.declare_dram_parameter("wqk", [DM, 2 * CW], BF16,
                                        isOutput=False)
    wv = nc.declare_dram_parameter("wv", [DM, CW], BF16, isOutput=False)
    wo = nc.declare_dram_parameter("wo", [CW, DM], BF16, isOutput=False)
    out = nc.declare_dram_parameter("out", [N, DM], BF16, isOutput=True)

    with tile.TileContext(nc) as tc:
        with (
            tc.tile_pool(name="cpool", bufs=1) as cpool,
            # 8 PSUM banks: "s" 2x[128,1024] scores, "av" 2x[128,512]
            # attention accumulators, "p1" 2x[128,512] projections + out-proj
            tc.tile_pool(name="psA", bufs=2, space="PSUM") as psA,
            tc.tile_pool(name="psB", bufs=2, space="PSUM") as psB,
            tc.tile_pool(name="psC", bufs=2, space="PSUM") as psC,
            tc.tile_pool(name="epool", bufs=5) as epool,
            tc.tile_pool(name="w1pool", bufs=1) as w1pool,
            tc.tile_pool(name="xpool", bufs=2) as xpool,
            tc.tile_pool(name="lpool", bufs=2) as lpool,
        ):
          for _rep in range(reps):
            qkT_t = [cpool.tile([128, N], BF16, name=f"qkT{m}") for m in range(MT)]
            # v tile: per head pair [v_even | ones | v_odd] (3*64 cols) -> the
            # fused av+rowsum matmul takes a contiguous [128, 128] lhsT for
            # either head, sharing the ones block; for the odd head the
            # output rows come out as [sums | out] instead of [out | sums]
            v_t = [cpool.tile([128, NPAIR * 3 * DH], BF16, name=f"v{j}")
                   for j in range(NJT)]
            aoT_t = [lpool.tile([128, N], BF16, name=f"aoT{c}", tag=f"aoT{c}",
                                bufs=1) for c in range(CW // 128)]
            wo_t = [lpool.tile([128, DM], BF16, name=f"wo{c}", tag=f"wo{c}",
                               bufs=1) for c in range(CW // 128)]
            if fp8_qk:
                wqk8_t = [w1pool.tile([128, 2 * 2 * CW], FP8, name=f"wqk8_{t}")
                          for t in range(KT // 2)]
            else:
                wqk_t = [w1pool.tile([128, 2 * CW], BF16, name=f"wqk{k}")
                         for k in range(KT)]
            wv_t = [w1pool.tile([128, CW], BF16, name=f"wv{k}")
                    for k in range(KT)]

            def scores_exp(p, ic, jt, tag="ex", bufs=5):
                """Score matmuls for one (pair, i-chunk, j-tile) + exp."""
                qt, kt = qkT_t[p], qkT_t[MT // 2 + p]
                isl = slice(ic * ICH, (ic + 1) * ICH)
                s_ps = psA.tile([128, 2 * ICH], F32, name="s_ps", tag="s")
                for half in range(2):
                    off = half * DH
                    nc.tensor.matmul(
                        s_ps[:, half * ICH:(half + 1) * ICH],
                        kt[off:off + DH, jt * 128:(jt + 1) * 128],
                        qt[off:off + DH, isl],
                        start=True, stop=True,
                    )
                ex = epool.tile([128, 2 * ICH], BF16, name=tag, tag=tag,
                                bufs=bufs)
                nc.scalar.activation(ex[:], s_ps[:], AF.Exp, scale=0.125)
                return ex

            class Stream:
                """One (pair, i-chunk) softmax-row accumulation with
                one-cell-ahead pipelining: scores of cell k+1 are emitted
                on PE before the (exp-dependent) av of cell k, so ScalarE
                is never behind a stalled av in PE program order."""

                def __init__(self, p, ic, av2, n_av):
                    self.p, self.ic, self.av2 = p, ic, av2
                    self.n_av = n_av       # total avs this stream will emit
                    self.done = 0
                    self.pend = None       # (jt, ex) awaiting its av

                def _av(self, jt, ex):
                    first, last = self.done == 0, self.done == self.n_av - 1
                    for half in range(2):
                        base = self.p * 3 * DH + half * DH
                        vl = v_t[jt][:, base:base + 2 * DH]
                        nc.tensor.matmul(
                            self.av2[half][:],
                            vl,
                            ex[:, half * ICH:(half + 1) * ICH],
                            start=first, stop=last,
                        )
                    self.done += 1

                def cell(self, jt, tag="ex", bufs=5):
                    ex = scores_exp(self.p, self.ic, jt, tag, bufs)
                    if self.pend is not None:
                        self._av(*self.pend)
                    self.pend = (jt, ex)

                def av_direct(self, jt, ex):
                    self._av(jt, ex)

                def flush(self):
                    if self.pend is not None:
                        self._av(*self.pend)
                        self.pend = None

            def normalize(p, ic, av2):
                isl = slice(ic * ICH, (ic + 1) * ICH)
                for half in range(2):
                    l = 2 * p + half
                    ct, coff = l // 2, (l % 2) * DH
                    # even head: rows [out | sums]; odd head: [sums | out]
                    o0, s0 = (0, DH) if half == 0 else (DH, 0)
                    rc = lpool.tile([DH, ICH], F32, name="rc", tag="rc", bufs=4)
                    nc.vector.reciprocal(rc[:], av2[half][s0:s0 + DH, :])
                    nc.vector.tensor_mul(
                        aoT_t[ct][coff:coff + DH, isl],
                        av2[half][o0:o0 + DH, :],
                        rc[:],
                    )

            def phase3_group(nt, h):
                po = psC.tile([128, 512], F32, name="po", tag="p1")
                for c in range(CW // 128):
                    nc.tensor.matmul(
                        po[:],
                        aoT_t[c][:, nt * 128:(nt + 1) * 128],
                        wo_t[c][:, h * 512:(h + 1) * 512],
                        start=(c == 0), stop=(c == CW // 128 - 1),
                    )
                os_ = lpool.tile([128, 512], BF16, name="os", tag="os")
                nc.vector.tensor_copy(os_[:], po[:])
                nc.gpsimd.dma_start(
                    out[nt * 128:(nt + 1) * 128, h * 512:(h + 1) * 512],
                    os_[:],
                )

            # ---------------- chunks + early attention ----------------
            av2_S0 = [psB.tile([128, ICH], F32, name=f"avS0_{h}", tag="av")
                      for h in range(2)]
            S0 = Stream(0, 0, av2_S0, NJT)
            emitted = {(p, ic): 0 for p in range(NPAIR) for ic in range(NIC)}
            defer_tiles = {}   # (p, ic, jt) -> parked ex tile
            defer_order = [(p, 0) for p in (1, 2, 3)] + \
                          [(p, 1) for p in range(NPAIR)]
            ndefer = 0

            for ch in range(N // NCH):
                csl = slice(ch * NCH, (ch + 1) * NCH)
                if fp8_qk:
                    x8_t = xpool.tile([128, (KT // 2) * 2 * NCH], FP8,
                                      name="x8_t", tag="x8")
                    nc.sync.dma_start(x8_t[:], x8[ch])
                if ch == 0:  # weights race ahead on the gpsimd+vector queues
                    if fp8_qk:
                        for t in range(KT // 2):
                            q = nc.gpsimd
                            q.dma_start(wqk8_t[t][:], wqk8[t])
                    else:
                        for k in range(KT):
                            q = nc.gpsimd
                            q.dma_start(wqk_t[k][:],
                                        wqk[k * 128:(k + 1) * 128, :])
                    for k in range(KT):
                        nc.gpsimd.dma_start(wv_t[k][:],
                                            wv[k * 128:(k + 1) * 128, :])
                x_t = xpool.tile([128, KT * NCH], BF16, name="x_t", tag="x")
                nc.sync.dma_start(x_t[:], xT[ch])
                if ch == 1:
                    for c in range(CW // 128):
                        nc.gpsimd.dma_start(wo_t[c][:],
                                            wo[c * 128:(c + 1) * 128, :])

                # proj work units for this chunk
                def qk_unit(m, x_t=x_t, csl=csl):
                    pq = psC.tile([128, NCH], F32, name="pq", tag="p1")
                    if fp8_qk:
                        w8v = [wqk8_t[t].rearrange("p (two m) -> p two m",
                                                   two=2)
                               for t in range(KT // 2)]
                        xr = x8_t.rearrange("p (t two n) -> p t two n",
                                            t=KT // 2, two=2)
                        for t in range(KT // 2):
                            nc.tensor.matmul(
                                pq[:],
                                w8v[t][:, :, m * 128:(m + 1) * 128],
                                xr[:, t, :, :],
                                start=(t == 0), stop=(t == KT // 2 - 1),
                                perf_mode=DR,
                            )
                    else:
                        for k in range(KT):
                            nc.tensor.matmul(
                                pq[:],
                                wqk_t[k][:, m * 128:(m + 1) * 128],
                                x_t[:, k * NCH:(k + 1) * NCH],
                                start=(k == 0), stop=(k == KT - 1),
                            )
                    nc.vector.tensor_copy(qkT_t[m][:, csl], pq[:])

                def v_unit(mt, x_t=x_t, ch=ch):
                    j = ch * (NCH // 128) + mt
                    pv = psC.tile([128, CW], F32, name="pv", tag="p1")
                    for k in range(KT):
                        nc.tensor.matmul(
                            pv[:],
                            x_t[:, k * NCH + mt * 128:k * NCH + (mt + 1) * 128],
                            wv_t[k][:],
                            start=(k == 0), stop=(k == KT - 1),
                        )
                    v3 = v_t[j].rearrange("p (q c) -> p q c", c=3 * DH)
                    pv3 = pv[:].rearrange("p (l c) -> p l c", c=DH)
                    nc.vector.tensor_copy(v3[:, :, 0:DH], pv3[:, 0::2, :])
                    nc.vector.tensor_copy(v3[:, :, 2 * DH:3 * DH],
                                          pv3[:, 1::2, :])
                    nc.any.memset(v3[:, :, DH:2 * DH], 1.0)

                last_ch = ch == N // NCH - 1
                if last_ch:
                    # final chunk: emit S0's kt tile + v tiles first so S0's
                    # last cells (which need this chunk's k/v) can interleave
                    morder = [MT // 2] + list(range(MT // 2 + 1, MT)) \
                        + list(range(MT // 2))
                    units = ([lambda m=morder[0]: qk_unit(m)]
                             + [lambda mt=mt: v_unit(mt)
                                for mt in range(NCH // 128)]
                             + [lambda m=m: qk_unit(m) for m in morder[1:]])
                else:
                    units = ([lambda m=m: qk_unit(m) for m in range(MT)]
                             + [lambda mt=mt: v_unit(mt)
                                for mt in range(NCH // 128)])

                # early-attention cells that unlock after the PREVIOUS chunk
                # (this chunk's outputs aren't ready mid-chunk): S0 staircases
                # with a live accumulator; more cells run scores+exp only and
                # park the exp for a later av replay
                cells = []
                jt_av = NJT if last_ch else min((NCH // 128) * ch, NJT)
                if ICH <= ch * NCH:
                    while emitted[(0, 0)] < jt_av:
                        jt = emitted[(0, 0)]
                        cells.append(lambda jt=jt: S0.cell(jt))
                        emitted[(0, 0)] += 1
                for (p, ic) in defer_order:
                    if ndefer >= DEFER:
                        break
                    if (ic + 1) * ICH > ch * NCH:
                        continue
                    while emitted[(p, ic)] < jt_av and ndefer < DEFER:
                        jt = emitted[(p, ic)]

                        def defer_cell(p=p, ic=ic, jt=jt):
                            defer_tiles[(p, ic, jt)] = scores_exp(
                                p, ic, jt, tag="exd", bufs=DEFER)

                        cells.append(defer_cell)
                        emitted[(p, ic)] += 1
                        ndefer += 1

                # interleave: proj units with cells spread evenly between them
                n_u, n_c = len(units), len(cells)
                ci = 0
                for ui, unit in enumerate(units):
                    unit()
                    want = (ui + 1) * n_c // n_u
                    while ci < want:
                        cells[ci]()
                        ci += 1
                while ci < n_c:
                    cells[ci]()
                    ci += 1

            # finish S0: all j-tiles now exist
            while emitted[(0, 0)] < NJT:
                S0.cell(emitted[(0, 0)])
                emitted[(0, 0)] += 1
            S0.flush()

            # ---------------- main loop (i-chunk-major) ----------------
            normalize(0, 0, av2_S0)
            p3_groups = []
            stream_i = 0
            for ic in range(NIC):
                for p in range(NPAIR):
                    if (p, ic) == (0, 0):
                        continue
                    # alternate accumulator PSUM tag so consecutive streams
                    # don't serialize; out-proj (psC "p1") only interleaves
                    # into psB streams
                    use_B = stream_i % 2 == 0
                    stream_i += 1
                    pool, tag = (psB, "av") if use_B else (psC, "p1")
                    av2 = [pool.tile([128, ICH], F32, name=f"av{h}", tag=tag)
                           for h in range(2)]
                    deferred = [jt for jt in range(NJT)
                                if (p, ic, jt) in defer_tiles]
                    fresh = [jt for jt in range(NJT)
                             if (p, ic, jt) not in defer_tiles]
                    st = Stream(p, ic, av2, NJT)
                    # fresh cells feed ScalarE (one-ahead); parked av replays
                    # and out-proj groups drop in as pure-PE filler 1-per-2
                    di = 0
                    for i, jt in enumerate(fresh):
                        st.cell(jt)
                        if i % 2 == 1:
                            if di < len(deferred):
                                jd = deferred[di]
                                di += 1
                                st.av_direct(jd, defer_tiles.pop((p, ic, jd)))
                            elif use_B and p3_groups:
                                phase3_group(*p3_groups.pop(0))
                    for jd in deferred[di:]:
                        st.av_direct(jd, defer_tiles.pop((p, ic, jd)))
                    st.flush()
                    normalize(p, ic, av2)
                p3_groups += [(nt, h)
                              for nt in range(ic * ICH // 128,
                                              (ic + 1) * ICH // 128)
                              for h in range(DM // 512)]
            while p3_groups:
                phase3_group(*p3_groups.pop(0))

    nc.finalize()
    return nc


def make_in_maps(x, w_qkv, w_out, fp8_qk=None):
    """Per-core input dict list (bf16 packing; shared by kernel() and
    test harnesses)."""
    if fp8_qk is None:
        fp8_qk = USE_FP8_QK
    bf = ml_dtypes.bfloat16
    f8 = mybir.dt.np(FP8)
    maps = []
    xp_cache = {}
    x8_cache = {}
    for core in range(8):
        b, g = divmod(core, 2)
        if b not in xp_cache:
            # pack x[b]^T as [chunk, partition, k, n] so device chunk loads
            # are single linear DMAs
            xp_cache[b] = np.ascontiguousarray(
                x[b].T.reshape(KT, 128, N // NCH, NCH).transpose(2, 1, 0, 3)
                .reshape(N // NCH, 128, KT * NCH).astype(bf))
            if fp8_qk:
                # [chunk, partition, t, pair, n] with d = t*256 + pair*128 + p
                x8_cache[b] = np.ascontiguousarray(
                    x[b].T.reshape(KT // 2, 2, 128, N // NCH, NCH)
                    .transpose(3, 2, 0, 1, 4)
                    .reshape(N // NCH, 128, (KT // 2) * 2 * NCH).astype(f8))
        wq = w_qkv[:, g * CW:(g + 1) * CW]
        wk = w_qkv[:, DM + g * CW:DM + (g + 1) * CW]
        wv_ = w_qkv[:, 2 * DM + g * CW:2 * DM + (g + 1) * CW]
        m = {
            "xT": xp_cache[b],
            "wv": np.ascontiguousarray(wv_.astype(bf)),
            "wo": np.ascontiguousarray(w_out[g * CW:(g + 1) * CW, :].astype(bf)),
        }
        wqk_full = np.concatenate([wq, wk], axis=1)
        if fp8_qk:
            m["x8"] = x8_cache[b]
            m["wqk8"] = np.ascontiguousarray(
                wqk_full.reshape(KT // 2, 2, 128, 2 * CW).transpose(0, 2, 1, 3)
                .reshape(KT // 2, 128, 2 * 2 * CW).astype(f8))
        else:
            m["wqk"] = np.ascontiguousarray(wqk_full.astype(bf))
        maps.append(m)
    return maps


_NC_CACHE = {}


def _get_nc():
    if "nc" not in _NC_CACHE:
        _NC_CACHE["nc"] = build_nc()
    return _NC_CACHE["nc"]


def kernel(x, w_qkv, w_out, b_out):
    x = np.ascontiguousarray(x, dtype=np.float32)
    w_qkv = np.asarray(w_qkv, dtype=np.float32)
    w_out = np.asarray(w_out, dtype=np.float32)
    b_out = np.asarray(b_out, dtype=np.float32)

    nc = _get_nc()
    in_maps = make_in_maps(x, w_qkv, w_out)
    res = run_bass_kernel_spmd(nc, in_maps, core_ids=list(range(8)))
    _NC_CACHE["last_result"] = res
    out = np.empty((B, N, DM), np.float32)
    for b in range(B):
        out[b] = (res.results[2 * b]["out"].astype(np.float32)
                  + res.results[2 * b + 1]["out"].astype(np.float32) + b_out)
    return out


# revision 24
# speedup vs baseline: 1.0234x; 1.0234x over previous
"""Multi-head attention on 8 Trainium2 NeuronCores.

Problem: x[4, 2048, 1024], 16 heads x 64 dim.
  qkv = x @ w_qkv; attn = softmax(q k^T / 8); out = (attn v) @ w_out + b_out

Sharding: 8 cores = 4 batches x 2 head-groups (8 heads each).
Each core computes a partial out-projection over its 8 heads' dims;
host sums the two partials per batch and adds the bias.

All SBUF operands are bf16 (halves DMA + SBUF vs the f32r version at the
same PE rate; scale-relative error ~2.6e-3 vs the fp32 reference). PSUM
accumulation stays fp32.

Per-core schedule (one merged pipeline, ScalarE exp is the scarce engine):
  chunks 0-7: qT,kT = (w_qk^T x^T) with d on partitions; v natural [n, 512]
    stored per head pair as [v_even | ones | v_odd] so the fused av matmul
    emits unnormalized out^T rows plus replicated softmax row-sums. After
    each chunk: (a) one full early attention stream (pair 0, ic 0)
    staircases through available j-tiles, (b) up to DEFER more cells run
    scores+exp only, parking exp tiles in SBUF so ScalarE stays busy while
    their av matmuls wait for PSUM accumulator slots.
  main loop, i-chunk-major: for ic, for pair: 16 (scores->exp->av) cells
    (deferred cells replay their parked exp tile, av-only), then the
    DVE reciprocal+mul normalize. Once all 4 pairs finish an ic, that
    i-chunk's out-projection groups are queued and interleaved 1-per-2-cells
    into later streams (only streams whose accumulators sit in psB, so the
    out-proj PSUM tag is free). Output is stored as bf16; the host sums the
    two partials per batch in fp32 and adds the bias.
"""

import numpy as np
import ml_dtypes

import concourse.bacc as bacc
import concourse.mybir as mybir
import concourse.tile as tile
from concourse.bass_utils import run_bass_kernel_spmd

F32 = mybir.dt.float32
BF16 = mybir.dt.bfloat16
FP8 = mybir.dt.float8e4
AF = mybir.ActivationFunctionType
DR = mybir.MatmulPerfMode.DoubleRow

B = 4          # batch
N = 2048       # sequence
DM = 1024      # model dim
NH = 16        # heads
DH = 64        # head dim
G = 2          # head groups (cores per batch)
HPC = NH // G  # heads per core = 8
CW = DH * HPC  # per-core qkv column width = 512

NCH = 512      # phase-1 x^T column chunk (double-chunk: q-granularity = ICH)
ICH = 512      # i (query) chunk (per head; a pair shares [128, 2*ICH])

KT = DM // 128      # 8 contraction tiles over d
MT = 2 * CW // 128  # 8 c-tiles for q|k
NJT = N // 128      # 16 j tiles
NIC = N // ICH      # 4 i chunks
NPAIR = HPC // 2    # 4 head pairs

DEFER = 26     # deferred score+exp cells parked in SBUF during chunks
USE_FP8_QK = True  # q/k projection via fp8e4 DoubleRow matmuls


def build_nc(reps=1, fp8_qk=None):
    if fp8_qk is None:
        fp8_qk = USE_FP8_QK
    nc = bacc.Bacc(None, target_bir_lowering=False, debug=False)

    # xT is host-packed to [chunk, partition, k*NCH] so every chunk load is
    # one fully linear 1 MB DMA
    xT = nc.declare_dram_parameter("xT", [N // NCH, 128, KT * NCH], BF16,
                                   isOutput=False)
    if fp8_qk:
        # fp8 copy of x for the DoubleRow q/k projection: per chunk
        # [128, t(4), pair(2), NCH] with d-slice pairing d = t*256+l*128+p.
        # N=512 of moving data per stationary keeps the (FWL-less) DoubleRow
        # LDWEIGHTS stream just under the matmul stream.
        x8 = nc.declare_dram_parameter(
            "x8", [N // NCH, 128, (KT // 2) * 2 * NCH], FP8, isOutput=False)
        wqk8 = nc.declare_dram_parameter(
            "wqk8", [KT // 2, 128, 2 * 2 * CW], FP8, isOutput=False)
    else:
        wqk = nc.declare_dram_parameter("wqk", [DM, 2 * CW], BF16,
                                        isOutput=False)
    wv = nc.declare_dram_parameter("wv", [DM, CW], BF16, isOutput=False)
    wo = nc.declare_dram_parameter("wo", [CW, DM], BF16, isOutput=False)
    out = nc.declare_dram_parameter("out", [N, DM], BF16, isOutput=True)

    with tile.TileContext(nc) as tc:
        with (
            tc.tile_pool(name="cpool", bufs=1) as cpool,
            # 8 PSUM banks: "s" 2x[128,1024] scores, "av" 2x[128,512]
            # attention accumulators, "p1" 2x[128,512] projections + out-proj
            tc.tile_pool(name="psA", bufs=2, space="PSUM") as psA,
            tc.tile_pool(name="psB", bufs=2, space="PSUM") as psB,
            tc.tile_pool(name="psC", bufs=2, space="PSUM") as psC,
            tc.tile_pool(name="epool", bufs=5) as epool,
            tc.tile_pool(name="w1pool", bufs=1) as w1pool,
            tc.tile_pool(name="xpool", bufs=3) as xpool,
            tc.tile_pool(name="lpool", bufs=2) as lpool,
        ):
          for _rep in range(reps):
            qkT_t = [cpool.tile([128, N], BF16, name=f"qkT{m}") for m in range(MT)]
            # v tile: per head pair [v_even | ones | v_odd] (3*64 cols) -> the
            # fused av+rowsum matmul takes a contiguous [128, 128] lhsT for
            # either head, sharing the ones block; for the odd head the
            # output rows come out as [sums | out] instead of [out | sums]
            v_t = [cpool.tile([128, NPAIR * 3 * DH], BF16, name=f"v{j}")
                   for j in range(NJT)]
            aoT_t = [lpool.tile([128, N], BF16, name=f"aoT{c}", tag=f"aoT{c}",
                                bufs=1) for c in range(CW // 128)]
            wo_t = [lpool.tile([128, DM], BF16, name=f"wo{c}", tag=f"wo{c}",
                               bufs=1) for c in range(CW // 128)]
            if fp8_qk:
                wqk8_t = [w1pool.tile([128, 2 * 2 * CW], FP8, name=f"wqk8_{t}")
                          for t in range(KT // 2)]
            else:
                wqk_t = [w1pool.tile([128, 2 * CW], BF16, name=f"wqk{k}")
                         for k in range(KT)]
            wv_t = [w1pool.tile([128, CW], BF16, name=f"wv{k}")
                    for k in range(KT)]

            def scores_exp(p, ic, jt, tag="ex", bufs=5):
                """Score matmuls for one (pair, i-chunk, j-tile) + exp."""
                qt, kt = qkT_t[p], qkT_t[MT // 2 + p]
                isl = slice(ic * ICH, (ic + 1) * ICH)
                s_ps = psA.tile([128, 2 * ICH], F32, name="s_ps", tag="s")
                for half in range(2):
                    off = half * DH
                    nc.tensor.matmul(
                        s_ps[:, half * ICH:(half + 1) * ICH],
                        kt[off:off + DH, jt * 128:(jt + 1) * 128],
                        qt[off:off + DH, isl],
                        start=True, stop=True,
                    )
                ex = epool.tile([128, 2 * ICH], BF16, name=tag, tag=tag,
                                bufs=bufs)
                nc.scalar.activation(ex[:], s_ps[:], AF.Exp, scale=0.125)
                return ex

            class Stream:
                """One (pair, i-chunk) softmax-row accumulation with
                one-cell-ahead pipelining: scores of cell k+1 are emitted
                on PE before the (exp-dependent) av of cell k, so ScalarE
                is never behind a stalled av in PE program order."""

                def __init__(self, p, ic, av2, n_av):
                    self.p, self.ic, self.av2 = p, ic, av2
                    self.n_av = n_av       # total avs this stream will emit
                    self.done = 0
                    self.pend = None       # (jt, ex) awaiting its av

                def _av(self, jt, ex):
                    first, last = self.done == 0, self.done == self.n_av - 1
                    for half in range(2):
                        base = self.p * 3 * DH + half * DH
                        vl = v_t[jt][:, base:base + 2 * DH]
                        nc.tensor.matmul(
                            self.av2[half][:],
                            vl,
                            ex[:, half * ICH:(half + 1) * ICH],
                            start=first, stop=last,
                        )
                    self.done += 1

                def cell(self, jt, tag="ex", bufs=5):
                    ex = scores_exp(self.p, self.ic, jt, tag, bufs)
                    if self.pend is not None:
                        self._av(*self.pend)
                    self.pend = (jt, ex)

                def av_direct(self, jt, ex):
                    self._av(jt, ex)

                def flush(self):
                    if self.pend is not None:
                        self._av(*self.pend)
                        self.pend = None

            def normalize(p, ic, av2):
                isl = slice(ic * ICH, (ic + 1) * ICH)
                for half in range(2):
                    l = 2 * p + half
                    ct, coff = l // 2, (l % 2) * DH
                    # even head: rows [out | sums]; odd head: [sums | out]
                    o0, s0 = (0, DH) if half == 0 else (DH, 0)
                    rc = lpool.tile([DH, ICH], F32, name="rc", tag="rc", bufs=4)
                    nc.vector.reciprocal(rc[:], av2[half][s0:s0 + DH, :])
                    nc.vector.tensor_mul(
                        aoT_t[ct][coff:coff + DH, isl],
                        av2[half][o0:o0 + DH, :],
                        rc[:],
                    )

            def phase3_group(nt, h):
                po = psC.tile([128, 512], F32, name="po", tag="p1")
                for c in range(CW // 128):
                    nc.tensor.matmul(
                        po[:],
                        aoT_t[c][:, nt * 128:(nt + 1) * 128],
                        wo_t[c][:, h * 512:(h + 1) * 512],
                        start=(c == 0), stop=(c == CW // 128 - 1),
                    )
                os_ = lpool.tile([128, 512], BF16, name="os", tag="os")
                nc.vector.tensor_copy(os_[:], po[:])
                nc.gpsimd.dma_start(
                    out[nt * 128:(nt + 1) * 128, h * 512:(h + 1) * 512],
                    os_[:],
                )

            # ---------------- chunks + early attention ----------------
            av2_S0 = [psB.tile([128, ICH], F32, name=f"avS0_{h}", tag="av")
                      for h in range(2)]
            S0 = Stream(0, 0, av2_S0, NJT)
            emitted = {(p, ic): 0 for p in range(NPAIR) for ic in range(NIC)}
            defer_tiles = {}   # (p, ic, jt) -> parked ex tile
            defer_order = [(p, 0) for p in (1, 2, 3)] + \
                          [(p, 1) for p in range(NPAIR)]
            ndefer = 0

            for ch in range(N // NCH):
                csl = slice(ch * NCH, (ch + 1) * NCH)
                if fp8_qk:
                    x8_t = xpool.tile([128, (KT // 2) * 2 * NCH], FP8,
                                      name="x8_t", tag="x8")
                    nc.sync.dma_start(x8_t[:], x8[ch])
                if ch == 0:  # weights race ahead on the gpsimd+vector queues
                    if fp8_qk:
                        for t in range(KT // 2):
                            q = nc.gpsimd
                            q.dma_start(wqk8_t[t][:], wqk8[t])
                    else:
                        for k in range(KT):
                            q = nc.gpsimd
                            q.dma_start(wqk_t[k][:],
                                        wqk[k * 128:(k + 1) * 128, :])
                    for k in range(KT):
                        nc.gpsimd.dma_start(wv_t[k][:],
                                            wv[k * 128:(k + 1) * 128, :])
                x_t = xpool.tile([128, KT * NCH], BF16, name="x_t", tag="x")
                nc.sync.dma_start(x_t[:], xT[ch])
                if ch == 1:
                    for c in range(CW // 128):
                        nc.gpsimd.dma_start(wo_t[c][:],
                                            wo[c * 128:(c + 1) * 128, :])

                # proj work units for this chunk
                def qk_unit(m, x_t=x_t, csl=csl):
                    pq = psC.tile([128, NCH], F32, name="pq", tag="p1")
                    if fp8_qk:
                        w8v = [wqk8_t[t].rearrange("p (two m) -> p two m",
                                                   two=2)
                               for t in range(KT // 2)]
                        xr = x8_t.rearrange("p (t two n) -> p t two n",
                                            t=KT // 2, two=2)
                        for t in range(KT // 2):
                            nc.tensor.matmul(
                                pq[:],
                                w8v[t][:, :, m * 128:(m + 1) * 128],
                                xr[:, t, :, :],
                                start=(t == 0), stop=(t == KT // 2 - 1),
                                perf_mode=DR,
                            )
                    else:
                        for k in range(KT):
                            nc.tensor.matmul(
                                pq[:],
                                wqk_t[k][:, m * 128:(m + 1) * 128],
                                x_t[:, k * NCH:(k + 1) * NCH],
                                start=(k == 0), stop=(k == KT - 1),
                            )
                    nc.vector.tensor_copy(qkT_t[m][:, csl], pq[:])

                def v_unit(mt, x_t=x_t, ch=ch):
                    j = ch * (NCH // 128) + mt
                    pv = psC.tile([128, CW], F32, name="pv", tag="p1")
                    for k in range(KT):
                        nc.tensor.matmul(
                            pv[:],
                            x_t[:, k * NCH + mt * 128:k * NCH + (mt + 1) * 128],
                            wv_t[k][:],
                            start=(k == 0), stop=(k == KT - 1),
                        )
                    v3 = v_t[j].rearrange("p (q c) -> p q c", c=3 * DH)
                    pv3 = pv[:].rearrange("p (l c) -> p l c", c=DH)
                    nc.vector.tensor_copy(v3[:, :, 0:DH], pv3[:, 0::2, :])
                    nc.vector.tensor_copy(v3[:, :, 2 * DH:3 * DH],
                                          pv3[:, 1::2, :])
                    nc.any.memset(v3[:, :, DH:2 * DH], 1.0)

                last_ch = ch == N // NCH - 1
                if last_ch:
                    # final chunk: emit S0's kt tile + v tiles first so S0's
                    # last cells (which need this chunk's k/v) can interleave
                    morder = [MT // 2] + list(range(MT // 2 + 1, MT)) \
                        + list(range(MT // 2))
                    units = ([lambda m=morder[0]: qk_unit(m)]
                             + [lambda mt=mt: v_unit(mt)
                                for mt in range(NCH // 128)]
                             + [lambda m=m: qk_unit(m) for m in morder[1:]])
                else:
                    units = ([lambda m=m: qk_unit(m) for m in range(MT)]
                             + [lambda mt=mt: v_unit(mt)
                                for mt in range(NCH // 128)])

                # early-attention cells that unlock after the PREVIOUS chunk
                # (this chunk's outputs aren't ready mid-chunk): S0 staircases
                # with a live accumulator; more cells run scores+exp only and
                # park the exp for a later av replay
                cells = []
                jt_av = NJT if last_ch else min((NCH // 128) * ch, NJT)
                if ICH <= ch * NCH:
                    while emitted[(0, 0)] < jt_av:
                        jt = emitted[(0, 0)]
                        cells.append(lambda jt=jt: S0.cell(jt))
                        emitted[(0, 0)] += 1
                for (p, ic) in defer_order:
                    if ndefer >= DEFER:
                        break
                    if (ic + 1) * ICH > ch * NCH:
                        continue
                    while emitted[(p, ic)] < jt_av and ndefer < DEFER:
                        jt = emitted[(p, ic)]

                        def defer_cell(p=p, ic=ic, jt=jt):
                            defer_tiles[(p, ic, jt)] = scores_exp(
                                p, ic, jt, tag="exd", bufs=DEFER)

                        cells.append(defer_cell)
                        emitted[(p, ic)] += 1
                        ndefer += 1

                # interleave: proj units with cells spread evenly between them
                n_u, n_c = len(units), len(cells)
                ci = 0
                for ui, unit in enumerate(units):
                    unit()
                    want = (ui + 1) * n_c // n_u
                    while ci < want:
                        cells[ci]()
                        ci += 1
                while ci < n_c:
                    cells[ci]()
                    ci += 1

            # finish S0: all j-tiles now exist
            while emitted[(0, 0)] < NJT:
                S0.cell(emitted[(0, 0)])
                emitted[(0, 0)] += 1
            S0.flush()

            # ---------------- main loop (i-chunk-major) ----------------
            normalize(0, 0, av2_S0)
            p3_groups = []
            stream_i = 0
            for ic in range(NIC):
                for p in range(NPAIR):
                    if (p, ic) == (0, 0):
                        continue
                    # alternate accumulator PSUM tag so consecutive streams
                    # don't serialize; out-proj (psC "p1") only interleaves
                    # into psB streams
                    use_B = stream_i % 2 == 0
                    stream_i += 1
                    pool, tag = (psB, "av") if use_B else (psC, "p1")
                    av2 = [pool.tile([128, ICH], F32, name=f"av{h}", tag=tag)
                           for h in range(2)]
                    deferred = [jt for jt in range(NJT)
                                if (p, ic, jt) in defer_tiles]
                    fresh = [jt for jt in range(NJT)
                             if (p, ic, jt) not in defer_tiles]
                    st = Stream(p, ic, av2, NJT)
                    # fresh cells feed ScalarE (one-ahead); parked av replays
                    # and out-proj groups drop in as pure-PE filler 1-per-2
                    di = 0
                    for i, jt in enumerate(fresh):
                        st.cell(jt)
                        if i % 2 == 1:
                            if di < len(deferred):
                                jd = deferred[di]
                                di += 1
                                st.av_direct(jd, defer_tiles.pop((p, ic, jd)))
                            elif use_B and p3_groups:
                                phase3_group(*p3_groups.pop(0))
                    for jd in deferred[di:]:
                        st.av_direct(jd, defer_tiles.pop((p, ic, jd)))
                    st.flush()
                    normalize(p, ic, av2)
                p3_groups += [(nt, h)
                              for nt in range(ic * ICH // 128,
                                              (ic + 1) * ICH // 128)
                              for h in range(DM // 512)]
            while p3_groups:
                phase3_group(*p3_groups.pop(0))

    nc.finalize()
    return nc


def make_in_maps(x, w_qkv, w_out, fp8_qk=None):
    """Per-core input dict list (bf16 packing; shared by kernel() and
    test harnesses)."""
    if fp8_qk is None:
        fp8_qk = USE_FP8_QK
    bf = ml_dtypes.bfloat16
    f8 = mybir.dt.np(FP8)
    maps = []
    xp_cache = {}
    x8_cache = {}
    for core in range(8):
        b, g = divmod(core, 2)
        if b not in xp_cache:
            # pack x[b]^T as [chunk, partition, k, n] so device chunk loads
            # are single linear DMAs
            xp_cache[b] = np.ascontiguousarray(
                x[b].T.reshape(KT, 128, N // NCH, NCH).transpose(2, 1, 0, 3)
                .reshape(N // NCH, 128, KT * NCH).astype(bf))
            if fp8_qk:
                # [chunk, partition, t, pair, n] with d = t*256 + pair*128 + p
                x8_cache[b] = np.ascontiguousarray(
                    x[b].T.reshape(KT // 2, 2, 128, N // NCH, NCH)
                    .transpose(3, 2, 0, 1, 4)
                    .reshape(N // NCH, 128, (KT // 2) * 2 * NCH).astype(f8))
        wq = w_qkv[:, g * CW:(g + 1) * CW]
        wk = w_qkv[:, DM + g * CW:DM + (g + 1) * CW]
        wv_ = w_qkv[:, 2 * DM + g * CW:2 * DM + (g + 1) * CW]
        m = {
            "xT": xp_cache[b],
            "wv": np.ascontiguousarray(wv_.astype(bf)),
            "wo": np.ascontiguousarray(w_out[g * CW:(g + 1) * CW, :].astype(bf)),
        }
        wqk_full = np.concatenate([wq, wk], axis=1)
        if fp8_qk:
            m["x8"] = x8_cache[b]
            m["wqk8"] = np.ascontiguousarray(
                wqk_full.reshape(KT // 2, 2, 128, 2 * CW).transpose(0, 2, 1, 3)
                .reshape(KT // 2, 128, 2 * 2 * CW).astype(f8))
        else:
            m["wqk"] = np.ascontiguousarray(wqk_full.astype(bf))
        maps.append(m)
    return maps


_NC_CACHE = {}


def _get_nc():
    if "nc" not in _NC_CACHE:
        _NC_CACHE["nc"] = build_nc()
    return _NC_CACHE["nc"]


def kernel(x, w_qkv, w_out, b_out):
    x = np.ascontiguousarray(x, dtype=np.float32)
    w_qkv = np.asarray(w_qkv, dtype=np.float32)
    w_out = np.asarray(w_out, dtype=np.float32)
    b_out = np.asarray(b_out, dtype=np.float32)

    nc = _get_nc()
    in_maps = make_in_maps(x, w_qkv, w_out)
    res = run_bass_kernel_spmd(nc, in_maps, core_ids=list(range(8)))
    _NC_CACHE["last_result"] = res
    out = np.empty((B, N, DM), np.float32)
    for b in range(B):
        out[b] = (res.results[2 * b]["out"].astype(np.float32)
                  + res.results[2 * b + 1]["out"].astype(np.float32) + b_out)
    return out


# revision 25
# speedup vs baseline: 1.2104x; 1.1827x over previous
"""Multi-head attention on 8 Trainium2 NeuronCores.

Problem: x[4, 2048, 1024], 16 heads x 64 dim.
  qkv = x @ w_qkv; attn = softmax(q k^T / 8); out = (attn v) @ w_out + b_out

Sharding: 8 cores = 4 batches x 2 head-groups (8 heads each).
Each core computes a partial out-projection over its 8 heads' dims;
host sums the two partials per batch and adds the bias.

All SBUF operands are bf16 (halves DMA + SBUF vs the f32r version at the
same PE rate; scale-relative error ~2.6e-3 vs the fp32 reference). PSUM
accumulation stays fp32.

Per-core schedule (one merged pipeline, ScalarE exp is the scarce engine):
  chunks 0-7: qT,kT = (w_qk^T x^T) with d on partitions; v natural [n, 512]
    stored per head pair as [v_even | ones | v_odd] so the fused av matmul
    emits unnormalized out^T rows plus replicated softmax row-sums. After
    each chunk: (a) one full early attention stream (pair 0, ic 0)
    staircases through available j-tiles, (b) up to DEFER more cells run
    scores+exp only, parking exp tiles in SBUF so ScalarE stays busy while
    their av matmuls wait for PSUM accumulator slots.
  main loop, i-chunk-major: for ic, for pair: 16 (scores->exp->av) cells
    (deferred cells replay their parked exp tile, av-only), then the
    DVE reciprocal+mul normalize. Once all 4 pairs finish an ic, that
    i-chunk's out-projection groups are queued and interleaved 1-per-2-cells
    into later streams (only streams whose accumulators sit in psB, so the
    out-proj PSUM tag is free). Output is stored as bf16; the host sums the
    two partials per batch in fp32 and adds the bias.
"""

import numpy as np
import ml_dtypes

import concourse.bacc as bacc
import concourse.mybir as mybir
import concourse.tile as tile
from concourse.bass_utils import run_bass_kernel_spmd

F32 = mybir.dt.float32
BF16 = mybir.dt.bfloat16
FP8 = mybir.dt.float8e4
AF = mybir.ActivationFunctionType
DR = mybir.MatmulPerfMode.DoubleRow

B = 4          # batch
N = 2048       # sequence
DM = 1024      # model dim
NH = 16        # heads
DH = 64        # head dim
G = 2          # head groups (cores per batch)
HPC = NH // G  # heads per core = 8
CW = DH * HPC  # per-core qkv column width = 512

NCH = 512      # phase-1 x^T column chunk (double-chunk: q-granularity = ICH)
ICH = 512      # i (query) chunk (per head; a pair shares [128, 2*ICH])

KT = DM // 128      # 8 contraction tiles over d
MT = 2 * CW // 128  # 8 c-tiles for q|k
NJT = N // 128      # 16 j tiles
NIC = N // ICH      # 4 i chunks
NPAIR = HPC // 2    # 4 head pairs

DEFER = 26     # deferred score+exp cells parked in SBUF during chunks
USE_FP8_QK = True  # q/k projection via fp8e4 DoubleRow matmuls


def build_nc(reps=1, fp8_qk=None):
    if fp8_qk is None:
        fp8_qk = USE_FP8_QK
    nc = bacc.Bacc(None, target_bir_lowering=False, debug=False)

    # xT is host-packed to [chunk, partition, k*NCH] so every chunk load is
    # one fully linear 1 MB DMA
    xT = nc.declare_dram_parameter("xT", [N // NCH, 128, KT * NCH], BF16,
                                   isOutput=False)
    if fp8_qk:
        # fp8 copy of x for the DoubleRow q/k projection: per chunk
        # [128, t(4), pair(2), NCH] with d-slice pairing d = t*256+l*128+p.
        # N=512 of moving data per stationary keeps the (FWL-less) DoubleRow
        # LDWEIGHTS stream just under the matmul stream.
        x8 = nc.declare_dram_parameter(
            "x8", [N // NCH, 128, (KT // 2) * 2 * NCH], FP8, isOutput=False)
        wqk8 = nc.declare_dram_parameter(
            "wqk8", [KT // 2, 128, 2 * 2 * CW], FP8, isOutput=False)
    else:
        wqk = nc.declare_dram_parameter("wqk", [DM, 2 * CW], BF16,
                                        isOutput=False)
    wv = nc.declare_dram_parameter("wv", [DM, CW], BF16, isOutput=False)
    wo = nc.declare_dram_parameter("wo", [CW, DM], BF16, isOutput=False)
    out = nc.declare_dram_parameter("out", [N, DM], BF16, isOutput=True)

    with tile.TileContext(nc) as tc:
        with (
            tc.tile_pool(name="cpool", bufs=1) as cpool,
            # 8 PSUM banks: "s" 2x[128,1024] scores, "av" 2x[128,512]
            # attention accumulators, "p1" 2x[128,512] projections + out-proj
            tc.tile_pool(name="psA", bufs=2, space="PSUM") as psA,
            tc.tile_pool(name="psB", bufs=2, space="PSUM") as psB,
            tc.tile_pool(name="psC", bufs=2, space="PSUM") as psC,
            tc.tile_pool(name="epool", bufs=5) as epool,
            tc.tile_pool(name="w1pool", bufs=1) as w1pool,
            tc.tile_pool(name="xpool", bufs=3) as xpool,
            tc.tile_pool(name="lpool", bufs=2) as lpool,
        ):
          for _rep in range(reps):
            qkT_t = [cpool.tile([128, N], BF16, name=f"qkT{m}") for m in range(MT)]
            # v tile: per head pair [v_even | ones | v_odd] (3*64 cols) -> the
            # fused av+rowsum matmul takes a contiguous [128, 128] lhsT for
            # either head, sharing the ones block; for the odd head the
            # output rows come out as [sums | out] instead of [out | sums]
            v_t = [cpool.tile([128, NPAIR * 3 * DH], BF16, name=f"v{j}")
                   for j in range(NJT)]
            aoT_t = [lpool.tile([128, N], BF16, name=f"aoT{c}", tag=f"aoT{c}",
                                bufs=1) for c in range(CW // 128)]
            wo_t = [lpool.tile([128, DM], BF16, name=f"wo{c}", tag=f"wo{c}",
                               bufs=1) for c in range(CW // 128)]
            if fp8_qk:
                wqk8_t = [w1pool.tile([128, 2 * 2 * CW], FP8, name=f"wqk8_{t}")
                          for t in range(KT // 2)]
            else:
                wqk_t = [w1pool.tile([128, 2 * CW], BF16, name=f"wqk{k}")
                         for k in range(KT)]
            wv_t = [w1pool.tile([128, CW], BF16, name=f"wv{k}")
                    for k in range(KT)]

            def scores_exp(p, ic, jt, tag="ex", bufs=5):
                """Score matmuls for one (pair, i-chunk, j-tile) + exp."""
                qt, kt = qkT_t[p], qkT_t[MT // 2 + p]
                isl = slice(ic * ICH, (ic + 1) * ICH)
                s_ps = psA.tile([128, 2 * ICH], F32, name="s_ps", tag="s")
                for half in range(2):
                    off = half * DH
                    nc.tensor.matmul(
                        s_ps[:, half * ICH:(half + 1) * ICH],
                        kt[off:off + DH, jt * 128:(jt + 1) * 128],
                        qt[off:off + DH, isl],
                        start=True, stop=True,
                    )
                ex = epool.tile([128, 2 * ICH], BF16, name=tag, tag=tag,
                                bufs=bufs)
                nc.scalar.activation(ex[:], s_ps[:], AF.Exp, scale=0.125)
                return ex

            class Stream:
                """One (pair, i-chunk) softmax-row accumulation with
                one-cell-ahead pipelining: scores of cell k+1 are emitted
                on PE before the (exp-dependent) av of cell k, so ScalarE
                is never behind a stalled av in PE program order."""

                def __init__(self, p, ic, av2, n_av):
                    self.p, self.ic, self.av2 = p, ic, av2
                    self.n_av = n_av       # total avs this stream will emit
                    self.done = 0
                    self.pend = None       # (jt, ex) awaiting its av

                def _av(self, jt, ex):
                    first, last = self.done == 0, self.done == self.n_av - 1
                    for half in range(2):
                        base = self.p * 3 * DH + half * DH
                        vl = v_t[jt][:, base:base + 2 * DH]
                        nc.tensor.matmul(
                            self.av2[half][:],
                            vl,
                            ex[:, half * ICH:(half + 1) * ICH],
                            start=first, stop=last,
                        )
                    self.done += 1

                def cell(self, jt, tag="ex", bufs=5):
                    ex = scores_exp(self.p, self.ic, jt, tag, bufs)
                    if self.pend is not None:
                        self._av(*self.pend)
                    self.pend = (jt, ex)

                def av_direct(self, jt, ex):
                    self._av(jt, ex)

                def flush(self):
                    if self.pend is not None:
                        self._av(*self.pend)
                        self.pend = None

            def normalize(p, ic, av2):
                isl = slice(ic * ICH, (ic + 1) * ICH)
                for half in range(2):
                    l = 2 * p + half
                    ct, coff = l // 2, (l % 2) * DH
                    # even head: rows [out | sums]; odd head: [sums | out]
                    o0, s0 = (0, DH) if half == 0 else (DH, 0)
                    rc = lpool.tile([DH, ICH], F32, name="rc", tag="rc", bufs=4)
                    nc.vector.reciprocal(rc[:], av2[half][s0:s0 + DH, :])
                    nc.vector.tensor_mul(
                        aoT_t[ct][coff:coff + DH, isl],
                        av2[half][o0:o0 + DH, :],
                        rc[:],
                    )

            def phase3_group(nt, h):
                po = psC.tile([128, 512], F32, name="po", tag="p1")
                for c in range(CW // 128):
                    nc.tensor.matmul(
                        po[:],
                        aoT_t[c][:, nt * 128:(nt + 1) * 128],
                        wo_t[c][:, h * 512:(h + 1) * 512],
                        start=(c == 0), stop=(c == CW // 128 - 1),
                    )
                os_ = lpool.tile([128, 512], BF16, name="os", tag="os")
                nc.vector.tensor_copy(os_[:], po[:])
                nc.gpsimd.dma_start(
                    out[nt * 128:(nt + 1) * 128, h * 512:(h + 1) * 512],
                    os_[:],
                )

            # ---------------- chunks + early attention ----------------
            av2_S0 = [psB.tile([128, ICH], F32, name=f"avS0_{h}", tag="av")
                      for h in range(2)]
            S0 = Stream(0, 0, av2_S0, NJT)
            emitted = {(p, ic): 0 for p in range(NPAIR) for ic in range(NIC)}
            defer_tiles = {}   # (p, ic, jt) -> parked ex tile
            defer_order = [(p, 0) for p in (1, 2, 3)] + \
                          [(p, 1) for p in range(NPAIR)]
            ndefer = 0

            for ch in range(N // NCH):
                csl = slice(ch * NCH, (ch + 1) * NCH)
                if fp8_qk:
                    x8_t = xpool.tile([128, (KT // 2) * 2 * NCH], FP8,
                                      name="x8_t", tag="x8")
                    nc.sync.dma_start(x8_t[:], x8[ch])
                if ch == 0:  # weights race ahead on the gpsimd+vector queues
                    if fp8_qk:
                        for t in range(KT // 2):
                            q = nc.gpsimd
                            q.dma_start(wqk8_t[t][:], wqk8[t])
                    else:
                        for k in range(KT):
                            q = nc.gpsimd
                            q.dma_start(wqk_t[k][:],
                                        wqk[k * 128:(k + 1) * 128, :])
                    for k in range(KT):
                        nc.gpsimd.dma_start(wv_t[k][:],
                                            wv[k * 128:(k + 1) * 128, :])
                x_t = xpool.tile([128, KT * NCH], BF16, name="x_t", tag="x")
                nc.sync.dma_start(x_t[:], xT[ch])
                if ch == 1:
                    for c in range(CW // 128):
                        nc.gpsimd.dma_start(wo_t[c][:],
                                            wo[c * 128:(c + 1) * 128, :])

                # proj work units for this chunk
                def qk_unit(m, x_t=x_t, csl=csl):
                    pq = psC.tile([128, NCH], F32, name="pq", tag="p1")
                    if fp8_qk:
                        w8v = [wqk8_t[t].rearrange("p (two m) -> p two m",
                                                   two=2)
                               for t in range(KT // 2)]
                        xr = x8_t.rearrange("p (t two n) -> p t two n",
                                            t=KT // 2, two=2)
                        for t in range(KT // 2):
                            nc.tensor.matmul(
                                pq[:],
                                w8v[t][:, :, m * 128:(m + 1) * 128],
                                xr[:, t, :, :],
                                start=(t == 0), stop=(t == KT // 2 - 1),
                                perf_mode=DR,
                            )
                    else:
                        for k in range(KT):
                            nc.tensor.matmul(
                                pq[:],
                                wqk_t[k][:, m * 128:(m + 1) * 128],
                                x_t[:, k * NCH:(k + 1) * NCH],
                                start=(k == 0), stop=(k == KT - 1),
                            )
                    nc.vector.tensor_copy(qkT_t[m][:, csl], pq[:])

                def v_unit(mt, x_t=x_t, ch=ch):
                    j = ch * (NCH // 128) + mt
                    pv = psC.tile([128, CW], F32, name="pv", tag="p1")
                    for k in range(KT):
                        nc.tensor.matmul(
                            pv[:],
                            x_t[:, k * NCH + mt * 128:k * NCH + (mt + 1) * 128],
                            wv_t[k][:],
                            start=(k == 0), stop=(k == KT - 1),
                        )
                    v3 = v_t[j].rearrange("p (q c) -> p q c", c=3 * DH)
                    pv3 = pv[:].rearrange("p (l c) -> p l c", c=DH)
                    nc.vector.tensor_copy(v3[:, :, 0:DH], pv3[:, 0::2, :])
                    nc.vector.tensor_copy(v3[:, :, 2 * DH:3 * DH],
                                          pv3[:, 1::2, :])
                    nc.any.memset(v3[:, :, DH:2 * DH], 1.0)

                units = ([lambda m=m: qk_unit(m) for m in range(MT)]
                         + [lambda mt=mt: v_unit(mt)
                            for mt in range(NCH // 128)])

                # early-attention cells that unlock after the PREVIOUS chunk
                # (this chunk's outputs aren't ready mid-chunk): S0 staircases
                # with a live accumulator; more cells run scores+exp only and
                # park the exp for a later av replay
                cells = []
                jt_av = min((NCH // 128) * ch, NJT)
                if ICH <= ch * NCH:
                    while emitted[(0, 0)] < jt_av:
                        jt = emitted[(0, 0)]
                        cells.append(lambda jt=jt: S0.cell(jt))
                        emitted[(0, 0)] += 1
                for (p, ic) in defer_order:
                    if ndefer >= DEFER:
                        break
                    if (ic + 1) * ICH > ch * NCH:
                        continue
                    while emitted[(p, ic)] < jt_av and ndefer < DEFER:
                        jt = emitted[(p, ic)]

                        def defer_cell(p=p, ic=ic, jt=jt):
                            defer_tiles[(p, ic, jt)] = scores_exp(
                                p, ic, jt, tag="exd", bufs=DEFER)

                        cells.append(defer_cell)
                        emitted[(p, ic)] += 1
                        ndefer += 1

                # interleave: proj units with cells spread evenly between them
                n_u, n_c = len(units), len(cells)
                ci = 0
                for ui, unit in enumerate(units):
                    unit()
                    want = (ui + 1) * n_c // n_u
                    while ci < want:
                        cells[ci]()
                        ci += 1
                while ci < n_c:
                    cells[ci]()
                    ci += 1

            # finish S0: all j-tiles now exist
            while emitted[(0, 0)] < NJT:
                S0.cell(emitted[(0, 0)])
                emitted[(0, 0)] += 1
            S0.flush()

            # ---------------- main loop (i-chunk-major) ----------------
            normalize(0, 0, av2_S0)
            p3_groups = []
            stream_i = 0
            for ic in range(NIC):
                for p in range(NPAIR):
                    if (p, ic) == (0, 0):
                        continue
                    # alternate accumulator PSUM tag so consecutive streams
                    # don't serialize; out-proj (psC "p1") only interleaves
                    # into psB streams
                    use_B = stream_i % 2 == 0
                    stream_i += 1
                    pool, tag = (psB, "av") if use_B else (psC, "p1")
                    av2 = [pool.tile([128, ICH], F32, name=f"av{h}", tag=tag)
                           for h in range(2)]
                    deferred = [jt for jt in range(NJT)
                                if (p, ic, jt) in defer_tiles]
                    fresh = [jt for jt in range(NJT)
                             if (p, ic, jt) not in defer_tiles]
                    st = Stream(p, ic, av2, NJT)
                    # fresh cells feed ScalarE (one-ahead); parked av replays
                    # and out-proj groups drop in as pure-PE filler 1-per-2
                    di = 0
                    for i, jt in enumerate(fresh):
                        st.cell(jt)
                        if i % 2 == 1:
                            if di < len(deferred):
                                jd = deferred[di]
                                di += 1
                                st.av_direct(jd, defer_tiles.pop((p, ic, jd)))
                            elif use_B and p3_groups:
                                phase3_group(*p3_groups.pop(0))
                    for jd in deferred[di:]:
                        st.av_direct(jd, defer_tiles.pop((p, ic, jd)))
                    st.flush()
                    normalize(p, ic, av2)
                p3_groups += [(nt, h)
                              for nt in range(ic * ICH // 128,
                                              (ic + 1) * ICH // 128)
                              for h in range(DM // 512)]
            while p3_groups:
                phase3_group(*p3_groups.pop(0))

    nc.finalize()
    return nc


def make_in_maps(x, w_qkv, w_out, fp8_qk=None):
    """Per-core input dict list (bf16 packing; shared by kernel() and
    test harnesses)."""
    if fp8_qk is None:
        fp8_qk = USE_FP8_QK
    bf = ml_dtypes.bfloat16
    f8 = mybir.dt.np(FP8)
    maps = []
    xp_cache = {}
    x8_cache = {}
    for core in range(8):
        b, g = divmod(core, 2)
        if b not in xp_cache:
            # pack x[b]^T as [chunk, partition, k, n] so device chunk loads
            # are single linear DMAs
            xp_cache[b] = np.ascontiguousarray(
                x[b].T.reshape(KT, 128, N // NCH, NCH).transpose(2, 1, 0, 3)
                .reshape(N // NCH, 128, KT * NCH).astype(bf))
            if fp8_qk:
                # [chunk, partition, t, pair, n] with d = t*256 + pair*128 + p
                x8_cache[b] = np.ascontiguousarray(
                    x[b].T.reshape(KT // 2, 2, 128, N // NCH, NCH)
                    .transpose(3, 2, 0, 1, 4)
                    .reshape(N // NCH, 128, (KT // 2) * 2 * NCH).astype(f8))
        wq = w_qkv[:, g * CW:(g + 1) * CW]
        wk = w_qkv[:, DM + g * CW:DM + (g + 1) * CW]
        wv_ = w_qkv[:, 2 * DM + g * CW:2 * DM + (g + 1) * CW]
        m = {
            "xT": xp_cache[b],
            "wv": np.ascontiguousarray(wv_.astype(bf)),
            "wo": np.ascontiguousarray(w_out[g * CW:(g + 1) * CW, :].astype(bf)),
        }
        wqk_full = np.concatenate([wq, wk], axis=1)
        if fp8_qk:
            m["x8"] = x8_cache[b]
            m["wqk8"] = np.ascontiguousarray(
                wqk_full.reshape(KT // 2, 2, 128, 2 * CW).transpose(0, 2, 1, 3)
                .reshape(KT // 2, 128, 2 * 2 * CW).astype(f8))
        else:
            m["wqk"] = np.ascontiguousarray(wqk_full.astype(bf))
        maps.append(m)
    return maps


_NC_CACHE = {}


def _get_nc():
    if "nc" not in _NC_CACHE:
        _NC_CACHE["nc"] = build_nc()
    return _NC_CACHE["nc"]


def kernel(x, w_qkv, w_out, b_out):
    x = np.ascontiguousarray(x, dtype=np.float32)
    w_qkv = np.asarray(w_qkv, dtype=np.float32)
    w_out = np.asarray(w_out, dtype=np.float32)
    b_out = np.asarray(b_out, dtype=np.float32)

    nc = _get_nc()
    in_maps = make_in_maps(x, w_qkv, w_out)
    res = run_bass_kernel_spmd(nc, in_maps, core_ids=list(range(8)))
    _NC_CACHE["last_result"] = res
    out = np.empty((B, N, DM), np.float32)
    for b in range(B):
        out[b] = (res.results[2 * b]["out"].astype(np.float32)
                  + res.results[2 * b + 1]["out"].astype(np.float32) + b_out)
    return out
